# revision 19
# baseline (speedup 1.0000x reference)
"""Trainium2 Bass kernel for nn_ContrastiveLoss (prototype InfoNCE loss).

Strategy (data-parallel over the N=100k cell axis, 8 NeuronCores):
  - Each core gets a 12544-row shard (rows padded with label=-1 / feat=0),
    laid out 98 contiguous rows per partition: row = p*98 + j.  Feature
    DMAs then move contiguous multi-KB spans per partition, and the labels
    land in [128, 98] layout directly -- no PE transpose needed.
  - Per tile j, a one-hot [128,64] matrix is built on-chip (DVE is_equal
    against an iota constant) and a bf16 matmul one_hot.T @ feat
    accumulates per-class sums into PSUM ([64, 256], fp32 accumulation).
    Features are cast f32->bf16 in-flight by the SWDGE DMA; the loss is
    insensitive to this rounding.  The stream runs at the per-core HBM
    f32-read roofline (~36 us for 25.7 MB).
  - The stream is ordered ALL-atac-chunks then ALL-rna-chunks, so the
    atac sums finish mid-stream and their PSUM->SBUF copy hides under
    the rna stream.  Post-stream device work is just the rna PSUM copy
    and one 128 KB output DMA.
  - Each core outputs its raw per-class partial sums [128, 256] f32
    (rows 0:64 atac, 64:128 rna).  The host reduces the 8 partials and
    computes the tiny K x K x D InfoNCE on the [64, 256] prototypes in
    float64 -- exact, and off the device critical path entirely (the
    sharding hint's AllReduce is replaced by the host gather that the
    full-I/O contract already requires).  Counts are never materialized:
    l2norm(sums/counts) == sums/||sums||.
"""
import sys

sys.path.insert(0, "/opt/trn_rl_repo")

import numpy as np
from contextlib import ExitStack

N, D, K = 100000, 256, 64
NCORES = 8
NTILES = 98               # tiles of 128 rows per core
NPAD = NTILES * 128       # 12544 rows per core (total 100352 >= 100000)
# Tapered chunk sizes: big chunks amortize DMA overhead; the tail
# shrinks so PE has almost no matmul backlog when the final bytes land.
CHUNKS = [32, 32, 24, 8, 2]
assert sum(CHUNKS) == NTILES
CHMAX = max(CHUNKS)
TAU = 0.5
EPS = 1e-8

_cache = {}


def _build(repeat_main=1, repeat_full=1, feat_fp8=False):
    import concourse.bacc as bacc
    import concourse.tile as tile
    from concourse import mybir

    f32, bf16, i32 = mybir.dt.float32, mybir.dt.bfloat16, mybir.dt.int32
    ft_dt = mybir.dt.float8e4 if feat_fp8 else bf16
    AF = mybir.ActivationFunctionType
    OP = mybir.AluOpType

    nc = bacc.Bacc(None, target_bir_lowering=False, debug=False,
                   num_devices=NCORES)

    fa = nc.dram_tensor("fa", [NPAD, D], f32, kind="ExternalInput")
    fr = nc.dram_tensor("fr", [NPAD, D], f32, kind="ExternalInput")
    la = nc.dram_tensor("la", [NPAD], i32, kind="ExternalInput")
    lr = nc.dram_tensor("lr", [NPAD], i32, kind="ExternalInput")
    # raw per-class partial sums; host reduces across cores and runs the
    # tiny [64, 256] InfoNCE tail in float64
    out = nc.dram_tensor("out", [128, D], f32, kind="ExternalOutput")

    iota_c = nc.inline_tensor(
        np.tile(np.arange(K, dtype=np.float32), (128, 1)), name="iota_c")

    with tile.TileContext(nc) as tc, ExitStack() as ctx:
        consts = ctx.enter_context(tc.tile_pool(name="consts", bufs=1))
        dram = ctx.enter_context(tc.tile_pool(name="dram", bufs=1,
                                              space="DRAM"))

        iota_sb = consts.tile([128, K], f32)
        nc.sync.dma_start(iota_sb, iota_c[:, :])

        def _body(tk_prev, make_tk):
            with tc.tile_pool(name="fin", bufs=1) as fin, \
                 tc.tile_pool(name="labels", bufs=1) as labels, \
                 tc.tile_pool(name="oh", bufs=1) as ohp, \
                 tc.tile_pool(name="feat", bufs=3) as featp, \
                 tc.tile_pool(name="psum_m", bufs=1, space="PSUM") as psum:

                # First atac chunk goes to the head of the gpsimd DMA
                # queue so HBM bytes start moving at t~0.
                ch0 = CHUNKS[0]
                ft0 = featp.tile([128, CHMAX, D], ft_dt,
                                 name="ft_a", tag="ft_a")
                if tk_prev is not None:
                    # bench-only serializer (repeat_full>1): tiny DMA
                    # reading rep k's output into the tile the first real
                    # DMA overwrites (WAW) -- orders rep k+1's stream
                    # behind rep k's tail.
                    nc.gpsimd.dma_start(ft0[0:1, 0:1, 0:1],
                                        tk_prev[0:1, 0:1])
                nc.gpsimd.dma_start(
                    ft0[:, :ch0, :],
                    fa[:, :].rearrange(
                        "(p j) e -> p j e", j=NTILES)[:, 0:ch0, :],
                )

                # labels: row p*98+j -> labT[p, j].  Loaded uncast (i32) on
                # the idle SP queue; DVE converts off the critical path.
                labT = {}
                for nm, lab in (("a", la), ("r", lr)):
                    li = labels.tile([128, NTILES], i32, name=f"labi_{nm}")
                    nc.sync.dma_start(
                        li, lab[:].rearrange("(p j) -> p j", j=NTILES))
                    lt = labels.tile([128, NTILES], f32, name=f"labT_{nm}")
                    nc.vector.tensor_copy(lt, li)
                    labT[nm] = lt

                # one-hots: oh[p, t, k] = (label[p*98+t] == k).  atac first
                # (split so chunk0's matmuls start early); rna built in two
                # just-in-time halves.
                oh = {}
                half = NTILES // 2
                for nm in ("a", "r"):
                    o = ohp.tile([128, NTILES, K], ft_dt,
                                 name=f"oh_{nm}")
                    parts = ((0, ch0), (ch0, NTILES)) if nm == "a" else \
                            ((0, half), (half, NTILES))
                    for lo, hi in parts:
                        w = hi - lo
                        nc.vector.tensor_tensor(
                            o[:, lo:hi, :],
                            iota_sb[:, None, :].to_broadcast([128, w, K]),
                            labT[nm][:, lo:hi, None].to_broadcast(
                                [128, w, K]),
                            OP.is_equal,
                        )
                    oh[nm] = o

                # Full-partition PSUM tiles so each accumulator owns its
                # bank at base_partition 0.
                psA_full = psum.tile([128, D], f32)
                psR_full = psum.tile([128, D], f32)
                ps = {"a": psA_full[0:K, :], "r": psR_full[0:K, :]}

                def _stream(nm, feat, first_ft):
                    for rep in range(repeat_main):
                        t0 = 0
                        for ci, w in enumerate(CHUNKS):
                            if ci == 0 and rep == 0 and first_ft is not None:
                                ft = first_ft
                            else:
                                ft = featp.tile([128, CHMAX, D], ft_dt,
                                                name=f"ft_{nm}",
                                                tag=f"ft_{nm}")
                                nc.gpsimd.dma_start(
                                    ft[:, :w, :],
                                    feat[:, :].rearrange(
                                        "(p j) e -> p j e",
                                        j=NTILES)[:, t0:t0 + w, :],
                                )
                            for j in range(w):
                                t = t0 + j
                                nc.tensor.matmul(ps[nm], oh[nm][:, t, :],
                                                 ft[:, j, :],
                                                 start=(t == 0),
                                                 stop=(t == NTILES - 1))
                            t0 += w

                _stream("a", fa, ft0)

                # atac PSUM copy AND its 64 KB out-DMA hide under the rna
                # stream (ACT engine + sync queue)
                outsb = fin.tile([128, D], f32)
                nc.scalar.activation(outsb[0:K, :], ps["a"], AF.Copy)
                nc.sync.dma_start(out[0:K, :], outsb[0:K, :])

                _stream("r", fr, None)

                # post-stream: one DVE copy + one 64 KB DMA out
                nc.vector.tensor_copy(outsb[K:128, :], ps["r"])
                nc.sync.dma_start(out[K:128, :], outsb[K:128, :])
                if make_tk:
                    tk = dram.tile([128, 1], f32)
                    nc.sync.dma_start(tk, outsb[:, 0:1])
                    return tk
            return None

        tk_prev = None
        for _full in range(repeat_full):
            tk_prev = _body(tk_prev, make_tk=(repeat_full > 1))

    nc.compile()
    return nc


def _get_nc(repeat_main=1, repeat_full=1, feat_fp8=False):
    key = ("nc", repeat_main, repeat_full, feat_fp8)
    if key not in _cache:
        _cache[key] = _build(repeat_main, repeat_full, feat_fp8)
    return _cache[key]


def _shard(arr, pad_value):
    """Split [N, ...] into NCORES shards of NPAD rows, padding the tail."""
    shards = []
    for i in range(NCORES):
        lo = min(i * NPAD, N)
        hi = min(lo + NPAD, N)
        part = arr[lo:hi]
        if part.shape[0] < NPAD:
            pad_shape = (NPAD - part.shape[0],) + arr.shape[1:]
            part = np.concatenate(
                [part, np.full(pad_shape, pad_value, dtype=arr.dtype)])
        shards.append(np.ascontiguousarray(part))
    return shards


def _shard_feat(arr):
    """[N, D] f32 -> NCORES shards of [NPAD, D] rows (zero-padded tail)."""
    return _shard(arr, 0.0)


def run_with_results(atac_feature, rna_feature, atac_label, rna_label,
                     **run_kwargs):
    from concourse import bass_utils

    nc = _get_nc()
    fa_s = _shard_feat(np.asarray(atac_feature, dtype=np.float32))
    fr_s = _shard_feat(np.asarray(rna_feature, dtype=np.float32))
    la_s = _shard(np.asarray(atac_label, dtype=np.int32), -1)
    lr_s = _shard(np.asarray(rna_label, dtype=np.int32), -1)
    in_maps = [
        {"fa": fa_s[i], "fr": fr_s[i], "la": la_s[i], "lr": lr_s[i]}
        for i in range(NCORES)
    ]
    return bass_utils.run_bass_kernel_spmd(
        nc, in_maps, core_ids=list(range(NCORES)), **run_kwargs)


def _host_tail(sums):
    """Exact [64, 256] InfoNCE tail in float64 on the reduced sums
    (rows 0:64 atac, 64:128 rna)."""
    A = sums[0:K]
    R = sums[K:128]
    A = A / np.maximum(np.sqrt((A * A).sum(1, keepdims=True)), 1e-12)
    R = R / np.maximum(np.sqrt((R * R).sum(1, keepdims=True)), 1e-12)

    Fp = np.exp(A * R / TAU)                               # [K, D]
    Sa = np.exp(A[:, None, :] * A[None, :, :] / TAU)       # [K, K, D]
    Sr = np.exp(A[:, None, :] * R[None, :, :] / TAU)
    off = (1.0 - np.eye(K))[:, :, None]
    Fn = ((Sa + Sr) * off).sum(axis=1) + 2.0 * (K - 1) * Fp
    loss_k = (-np.log(Fp / (Fn + EPS))).mean(axis=1)
    return loss_k.sum()


def kernel(atac_feature, rna_feature, atac_label, rna_label):
    res = run_with_results(atac_feature, rna_feature, atac_label, rna_label)
    sums = np.zeros((128, D), dtype=np.float64)
    for r in res.results:
        sums += np.asarray(r["out"], dtype=np.float64)
    return np.float32(_host_tail(sums))


# revision 20
# speedup vs baseline: 1.0540x; 1.0540x over previous
"""Trainium2 Bass kernel for nn_ContrastiveLoss (prototype InfoNCE loss).

Strategy (data-parallel over the N=100k cell axis, 8 NeuronCores):
  - Each core gets a 12544-row shard (rows padded with label=-1 / feat=0),
    laid out 98 contiguous rows per partition: row = p*98 + j.  Feature
    DMAs then move contiguous multi-KB spans per partition, and the labels
    land in [128, 98] layout directly -- no PE transpose needed.
  - Per tile j, a one-hot [128,64] matrix is built on-chip (DVE is_equal
    against an iota constant) and a bf16 matmul one_hot.T @ feat
    accumulates per-class sums into PSUM ([64, 256], fp32 accumulation).
    Features are cast f32->bf16 in-flight by the SWDGE DMA; the loss is
    insensitive to this rounding.  The stream runs at the per-core HBM
    f32-read roofline (~36 us for 25.7 MB).
  - The stream is ordered ALL-atac-chunks then ALL-rna-chunks, so the
    atac sums finish mid-stream and their PSUM->SBUF copy hides under
    the rna stream.  Post-stream device work is just the rna PSUM copy
    and one 128 KB output DMA.
  - Each core outputs its raw per-class partial sums [128, 256] f32
    (rows 0:64 atac, 64:128 rna).  The host reduces the 8 partials and
    computes the tiny K x K x D InfoNCE on the [64, 256] prototypes in
    float64 -- exact, and off the device critical path entirely (the
    sharding hint's AllReduce is replaced by the host gather that the
    full-I/O contract already requires).  Counts are never materialized:
    l2norm(sums/counts) == sums/||sums||.
"""
import sys

sys.path.insert(0, "/opt/trn_rl_repo")

import numpy as np
from contextlib import ExitStack

N, D, K = 100000, 256, 64
NCORES = 8
NTILES = 98               # tiles of 128 rows per core
NPAD = NTILES * 128       # 12544 rows per core (total 100352 >= 100000)
# Tapered chunk sizes: big chunks amortize DMA overhead; the tail
# shrinks so PE has almost no matmul backlog when the final bytes land.
CHUNKS = [32, 32, 24, 8, 1, 1]
assert sum(CHUNKS) == NTILES
CHMAX = max(CHUNKS)
TAU = 0.5
EPS = 1e-8

_cache = {}


def _build(repeat_main=1, repeat_full=1, feat_fp8=False):
    import concourse.bacc as bacc
    import concourse.tile as tile
    from concourse import mybir

    f32, bf16, i32 = mybir.dt.float32, mybir.dt.bfloat16, mybir.dt.int32
    ft_dt = mybir.dt.float8e4 if feat_fp8 else bf16
    AF = mybir.ActivationFunctionType
    OP = mybir.AluOpType

    nc = bacc.Bacc(None, target_bir_lowering=False, debug=False,
                   num_devices=NCORES)

    fa = nc.dram_tensor("fa", [NPAD, D], f32, kind="ExternalInput")
    fr = nc.dram_tensor("fr", [NPAD, D], f32, kind="ExternalInput")
    la = nc.dram_tensor("la", [NPAD], i32, kind="ExternalInput")
    lr = nc.dram_tensor("lr", [NPAD], i32, kind="ExternalInput")
    # raw per-class partial sums; host reduces across cores and runs the
    # tiny [64, 256] InfoNCE tail in float64
    out = nc.dram_tensor("out", [128, D], f32, kind="ExternalOutput")

    iota_c = nc.inline_tensor(
        np.tile(np.arange(K, dtype=np.float32), (128, 1)), name="iota_c")

    with tile.TileContext(nc) as tc, ExitStack() as ctx:
        consts = ctx.enter_context(tc.tile_pool(name="consts", bufs=1))
        dram = ctx.enter_context(tc.tile_pool(name="dram", bufs=1,
                                              space="DRAM"))

        iota_sb = consts.tile([128, K], f32)
        nc.sync.dma_start(iota_sb, iota_c[:, :])

        def _body(tk_prev, make_tk):
            with tc.tile_pool(name="fin", bufs=1) as fin, \
                 tc.tile_pool(name="labels", bufs=1) as labels, \
                 tc.tile_pool(name="oh", bufs=1) as ohp, \
                 tc.tile_pool(name="feat", bufs=3) as featp, \
                 tc.tile_pool(name="psum_m", bufs=1, space="PSUM") as psum:

                # First atac chunk goes to the head of the gpsimd DMA
                # queue so HBM bytes start moving at t~0.
                ch0 = CHUNKS[0]
                ft0 = featp.tile([128, CHMAX, D], ft_dt,
                                 name="ft_a", tag="ft_a")
                if tk_prev is not None:
                    # bench-only serializer (repeat_full>1): tiny DMA
                    # reading rep k's output into the tile the first real
                    # DMA overwrites (WAW) -- orders rep k+1's stream
                    # behind rep k's tail.
                    nc.gpsimd.dma_start(ft0[0:1, 0:1, 0:1],
                                        tk_prev[0:1, 0:1])
                nc.gpsimd.dma_start(
                    ft0[:, :ch0, :],
                    fa[:, :].rearrange(
                        "(p j) e -> p j e", j=NTILES)[:, 0:ch0, :],
                )

                # labels: row p*98+j -> labT[p, j].  Loaded uncast (i32) on
                # the idle SP queue; DVE converts off the critical path.
                labT = {}
                for nm, lab in (("a", la), ("r", lr)):
                    li = labels.tile([128, NTILES], i32, name=f"labi_{nm}")
                    nc.sync.dma_start(
                        li, lab[:].rearrange("(p j) -> p j", j=NTILES))
                    lt = labels.tile([128, NTILES], f32, name=f"labT_{nm}")
                    nc.vector.tensor_copy(lt, li)
                    labT[nm] = lt

                # one-hots: oh[p, t, k] = (label[p*98+t] == k).  atac first
                # (split so chunk0's matmuls start early); rna built in two
                # just-in-time halves.
                oh = {}
                half = NTILES // 2
                for nm in ("a", "r"):
                    o = ohp.tile([128, NTILES, K], ft_dt,
                                 name=f"oh_{nm}")
                    parts = ((0, ch0), (ch0, NTILES)) if nm == "a" else \
                            ((0, half), (half, NTILES))
                    for lo, hi in parts:
                        w = hi - lo
                        nc.vector.tensor_tensor(
                            o[:, lo:hi, :],
                            iota_sb[:, None, :].to_broadcast([128, w, K]),
                            labT[nm][:, lo:hi, None].to_broadcast(
                                [128, w, K]),
                            OP.is_equal,
                        )
                    oh[nm] = o

                # Full-partition PSUM tiles so each accumulator owns its
                # bank at base_partition 0.
                psA_full = psum.tile([128, D], f32)
                psR_full = psum.tile([128, D], f32)
                ps = {"a": psA_full[0:K, :], "r": psR_full[0:K, :]}

                def _stream(nm, feat, first_ft):
                    for rep in range(repeat_main):
                        t0 = 0
                        for ci, w in enumerate(CHUNKS):
                            if ci == 0 and rep == 0 and first_ft is not None:
                                ft = first_ft
                            else:
                                ft = featp.tile([128, CHMAX, D], ft_dt,
                                                name=f"ft_{nm}",
                                                tag=f"ft_{nm}")
                                nc.gpsimd.dma_start(
                                    ft[:, :w, :],
                                    feat[:, :].rearrange(
                                        "(p j) e -> p j e",
                                        j=NTILES)[:, t0:t0 + w, :],
                                )
                            for j in range(w):
                                t = t0 + j
                                nc.tensor.matmul(ps[nm], oh[nm][:, t, :],
                                                 ft[:, j, :],
                                                 start=(t == 0),
                                                 stop=(t == NTILES - 1))
                            t0 += w

                _stream("a", fa, ft0)

                # atac PSUM copy AND its 64 KB out-DMA hide under the rna
                # stream (ACT engine + sync queue)
                outsb = fin.tile([128, D], f32)
                nc.scalar.activation(outsb[0:K, :], ps["a"], AF.Copy)
                nc.sync.dma_start(out[0:K, :], outsb[0:K, :])

                _stream("r", fr, None)

                # post-stream: the rna PSUM copy split across the two
                # idle engines (column halves), then one 64 KB DMA out
                nc.vector.tensor_copy(outsb[K:128, 0:128],
                                      ps["r"][:, 0:128])
                nc.scalar.activation(outsb[K:128, 128:256],
                                     ps["r"][:, 128:256], AF.Copy)
                nc.sync.dma_start(out[K:128, :], outsb[K:128, :])
                if make_tk:
                    tk = dram.tile([128, 1], f32)
                    nc.sync.dma_start(tk, outsb[:, 0:1])
                    return tk
            return None

        tk_prev = None
        for _full in range(repeat_full):
            tk_prev = _body(tk_prev, make_tk=(repeat_full > 1))

    nc.compile()
    return nc


def _get_nc(repeat_main=1, repeat_full=1, feat_fp8=False):
    key = ("nc", repeat_main, repeat_full, feat_fp8)
    if key not in _cache:
        _cache[key] = _build(repeat_main, repeat_full, feat_fp8)
    return _cache[key]


def _shard(arr, pad_value):
    """Split [N, ...] into NCORES shards of NPAD rows, padding the tail."""
    shards = []
    for i in range(NCORES):
        lo = min(i * NPAD, N)
        hi = min(lo + NPAD, N)
        part = arr[lo:hi]
        if part.shape[0] < NPAD:
            pad_shape = (NPAD - part.shape[0],) + arr.shape[1:]
            part = np.concatenate(
                [part, np.full(pad_shape, pad_value, dtype=arr.dtype)])
        shards.append(np.ascontiguousarray(part))
    return shards


def _shard_feat(arr):
    """[N, D] f32 -> NCORES shards of [NPAD, D] rows (zero-padded tail)."""
    return _shard(arr, 0.0)


def run_with_results(atac_feature, rna_feature, atac_label, rna_label,
                     **run_kwargs):
    from concourse import bass_utils

    nc = _get_nc()
    fa_s = _shard_feat(np.asarray(atac_feature, dtype=np.float32))
    fr_s = _shard_feat(np.asarray(rna_feature, dtype=np.float32))
    la_s = _shard(np.asarray(atac_label, dtype=np.int32), -1)
    lr_s = _shard(np.asarray(rna_label, dtype=np.int32), -1)
    in_maps = [
        {"fa": fa_s[i], "fr": fr_s[i], "la": la_s[i], "lr": lr_s[i]}
        for i in range(NCORES)
    ]
    return bass_utils.run_bass_kernel_spmd(
        nc, in_maps, core_ids=list(range(NCORES)), **run_kwargs)


def _host_tail(sums):
    """Exact [64, 256] InfoNCE tail in float64 on the reduced sums
    (rows 0:64 atac, 64:128 rna)."""
    A = sums[0:K]
    R = sums[K:128]
    A = A / np.maximum(np.sqrt((A * A).sum(1, keepdims=True)), 1e-12)
    R = R / np.maximum(np.sqrt((R * R).sum(1, keepdims=True)), 1e-12)

    Fp = np.exp(A * R / TAU)                               # [K, D]
    Sa = np.exp(A[:, None, :] * A[None, :, :] / TAU)       # [K, K, D]
    Sr = np.exp(A[:, None, :] * R[None, :, :] / TAU)
    off = (1.0 - np.eye(K))[:, :, None]
    Fn = ((Sa + Sr) * off).sum(axis=1) + 2.0 * (K - 1) * Fp
    loss_k = (-np.log(Fp / (Fn + EPS))).mean(axis=1)
    return loss_k.sum()


def kernel(atac_feature, rna_feature, atac_label, rna_label):
    res = run_with_results(atac_feature, rna_feature, atac_label, rna_label)
    sums = np.zeros((128, D), dtype=np.float64)
    for r in res.results:
        sums += np.asarray(r["out"], dtype=np.float64)
    return np.float32(_host_tail(sums))


# revision 21
# speedup vs baseline: 1.0771x; 1.0219x over previous
"""Trainium2 Bass kernel for nn_ContrastiveLoss (prototype InfoNCE loss).

Strategy (data-parallel over the N=100k cell axis, 8 NeuronCores):
  - Each core gets a 12544-row shard (rows padded with label=-1 / feat=0),
    laid out 98 contiguous rows per partition: row = p*98 + j.  Feature
    DMAs then move contiguous multi-KB spans per partition, and the labels
    land in [128, 98] layout directly -- no PE transpose needed.
  - Per tile j, a one-hot [128,64] matrix is built on-chip (DVE is_equal
    against an iota constant) and a bf16 matmul one_hot.T @ feat
    accumulates per-class sums into PSUM ([64, 256], fp32 accumulation).
    Features are cast f32->bf16 in-flight by the SWDGE DMA; the loss is
    insensitive to this rounding.  The stream runs at the per-core HBM
    f32-read roofline (~36 us for 25.7 MB).
  - The stream is ordered ALL-atac-chunks then ALL-rna-chunks, so the
    atac sums finish mid-stream and their PSUM->SBUF copy hides under
    the rna stream.  Post-stream device work is just the rna PSUM copy
    and one 128 KB output DMA.
  - Each core outputs its raw per-class partial sums [128, 256] f32
    (rows 0:64 atac, 64:128 rna).  The host reduces the 8 partials and
    computes the tiny K x K x D InfoNCE on the [64, 256] prototypes in
    float64 -- exact, and off the device critical path entirely (the
    sharding hint's AllReduce is replaced by the host gather that the
    full-I/O contract already requires).  Counts are never materialized:
    l2norm(sums/counts) == sums/||sums||.
"""
import sys

sys.path.insert(0, "/opt/trn_rl_repo")

import numpy as np
from contextlib import ExitStack

N, D, K = 100000, 256, 64
NCORES = 8
NTILES = 98               # tiles of 128 rows per core
NPAD = NTILES * 128       # 12544 rows per core (total 100352 >= 100000)
# Tapered chunk sizes: big chunks amortize DMA overhead; the tail
# shrinks so PE has almost no matmul backlog when the final bytes land.
CHUNKS = [32, 32, 24, 8, 1, 1]
assert sum(CHUNKS) == NTILES
CHMAX = max(CHUNKS)
TAU = 0.5
EPS = 1e-8

_cache = {}


def _build(repeat_main=1, repeat_full=1, feat_fp8=False):
    import concourse.bacc as bacc
    import concourse.tile as tile
    from concourse import mybir

    f32, bf16, i32 = mybir.dt.float32, mybir.dt.bfloat16, mybir.dt.int32
    ft_dt = mybir.dt.float8e4 if feat_fp8 else bf16
    AF = mybir.ActivationFunctionType
    OP = mybir.AluOpType

    nc = bacc.Bacc(None, target_bir_lowering=False, debug=False,
                   num_devices=NCORES)

    fa = nc.dram_tensor("fa", [NPAD, D], f32, kind="ExternalInput")
    fr = nc.dram_tensor("fr", [NPAD, D], f32, kind="ExternalInput")
    la = nc.dram_tensor("la", [NPAD], i32, kind="ExternalInput")
    lr = nc.dram_tensor("lr", [NPAD], i32, kind="ExternalInput")
    # raw per-class partial sums; host reduces across cores and runs the
    # tiny [64, 256] InfoNCE tail in float64
    out = nc.dram_tensor("out", [128, D], f32, kind="ExternalOutput")

    iota_c = nc.inline_tensor(
        np.tile(np.arange(K, dtype=np.float32), (128, 1)), name="iota_c")

    with tile.TileContext(nc) as tc, ExitStack() as ctx:
        consts = ctx.enter_context(tc.tile_pool(name="consts", bufs=1))
        dram = ctx.enter_context(tc.tile_pool(name="dram", bufs=1,
                                              space="DRAM"))

        iota_sb = consts.tile([128, K], f32)
        nc.sync.dma_start(iota_sb, iota_c[:, :])

        def _body(tk_prev, make_tk):
            with tc.tile_pool(name="fin", bufs=1) as fin, \
                 tc.tile_pool(name="labels", bufs=1) as labels, \
                 tc.tile_pool(name="oh", bufs=1) as ohp, \
                 tc.tile_pool(name="feat", bufs=3) as featp, \
                 tc.tile_pool(name="psum_m", bufs=1, space="PSUM") as psum:

                # First atac chunk goes to the head of the gpsimd DMA
                # queue so HBM bytes start moving at t~0.
                ch0 = CHUNKS[0]
                ft0 = featp.tile([128, CHMAX, D], ft_dt,
                                 name="ft_a", tag="ft_a")
                if tk_prev is not None:
                    # bench-only serializer (repeat_full>1): tiny DMA
                    # reading rep k's output into the tile the first real
                    # DMA overwrites (WAW) -- orders rep k+1's stream
                    # behind rep k's tail.
                    nc.gpsimd.dma_start(ft0[0:1, 0:1, 0:1],
                                        tk_prev[0:1, 0:1])
                nc.gpsimd.dma_start(
                    ft0[:, :ch0, :],
                    fa[:, :].rearrange(
                        "(p j) e -> p j e", j=NTILES)[:, 0:ch0, :],
                )

                # labels: row p*98+j -> labT[p, j].  Loaded uncast (i32) on
                # the idle SP queue; DVE converts off the critical path.
                labT = {}
                for nm, lab in (("a", la), ("r", lr)):
                    li = labels.tile([128, NTILES], i32, name=f"labi_{nm}")
                    nc.sync.dma_start(
                        li, lab[:].rearrange("(p j) -> p j", j=NTILES))
                    lt = labels.tile([128, NTILES], f32, name=f"labT_{nm}")
                    nc.vector.tensor_copy(lt, li)
                    labT[nm] = lt

                # one-hots: oh[p, t, k] = (label[p*98+t] == k).  atac first
                # (split so chunk0's matmuls start early); rna built in two
                # just-in-time halves.
                oh = {}
                half = NTILES // 2
                for nm in ("a", "r"):
                    o = ohp.tile([128, NTILES, K], ft_dt,
                                 name=f"oh_{nm}")
                    parts = ((0, ch0), (ch0, NTILES)) if nm == "a" else \
                            ((0, half), (half, NTILES))
                    for lo, hi in parts:
                        w = hi - lo
                        nc.vector.tensor_tensor(
                            o[:, lo:hi, :],
                            iota_sb[:, None, :].to_broadcast([128, w, K]),
                            labT[nm][:, lo:hi, None].to_broadcast(
                                [128, w, K]),
                            OP.is_equal,
                        )
                    oh[nm] = o

                # Full-partition PSUM tiles so each accumulator owns its
                # bank at base_partition 0.
                psA_full = psum.tile([128, D], f32)
                psR_full = psum.tile([128, D], f32)
                ps = {"a": psA_full[0:K, :], "r": psR_full[0:K, :]}

                def _stream(nm, feat, first_ft):
                    for rep in range(repeat_main):
                        t0 = 0
                        for ci, w in enumerate(CHUNKS):
                            if ci == 0 and rep == 0 and first_ft is not None:
                                ft = first_ft
                            else:
                                ft = featp.tile([128, CHMAX, D], ft_dt,
                                                name=f"ft_{nm}",
                                                tag=f"ft_{nm}")
                                nc.gpsimd.dma_start(
                                    ft[:, :w, :],
                                    feat[:, :].rearrange(
                                        "(p j) e -> p j e",
                                        j=NTILES)[:, t0:t0 + w, :],
                                )
                            for j in range(w):
                                t = t0 + j
                                nc.tensor.matmul(ps[nm], oh[nm][:, t, :],
                                                 ft[:, j, :],
                                                 start=(t == 0),
                                                 stop=(t == NTILES - 1))
                            t0 += w

                _stream("a", fa, ft0)

                # atac PSUM copy AND its 64 KB out-DMA hide under the rna
                # stream (ACT engine + sync queue)
                outsb = fin.tile([128, D], f32)
                nc.scalar.activation(outsb[0:K, :], ps["a"], AF.Copy)
                nc.sync.dma_start(out[0:K, :], outsb[0:K, :])

                _stream("r", fr, None)

                # post-stream: the rna PSUM copy split across the two
                # idle engines (column halves), each half flushed on its
                # own HWDGE ring (qSPDynamicHW / qActDynamicHW) so the two
                # 32 KB out-DMAs and their completion receipts overlap
                nc.vector.tensor_copy(outsb[K:128, 0:128],
                                      ps["r"][:, 0:128])
                nc.scalar.activation(outsb[K:128, 128:256],
                                     ps["r"][:, 128:256], AF.Copy)
                nc.sync.dma_start(out[K:128, 0:128],
                                  outsb[K:128, 0:128])
                nc.scalar.dma_start(out[K:128, 128:256],
                                    outsb[K:128, 128:256])
                if make_tk:
                    tk = dram.tile([128, 1], f32)
                    nc.sync.dma_start(tk, outsb[:, 0:1])
                    return tk
            return None

        tk_prev = None
        for _full in range(repeat_full):
            tk_prev = _body(tk_prev, make_tk=(repeat_full > 1))

    nc.compile()
    return nc


def _get_nc(repeat_main=1, repeat_full=1, feat_fp8=False):
    key = ("nc", repeat_main, repeat_full, feat_fp8)
    if key not in _cache:
        _cache[key] = _build(repeat_main, repeat_full, feat_fp8)
    return _cache[key]


def _shard(arr, pad_value):
    """Split [N, ...] into NCORES shards of NPAD rows, padding the tail."""
    shards = []
    for i in range(NCORES):
        lo = min(i * NPAD, N)
        hi = min(lo + NPAD, N)
        part = arr[lo:hi]
        if part.shape[0] < NPAD:
            pad_shape = (NPAD - part.shape[0],) + arr.shape[1:]
            part = np.concatenate(
                [part, np.full(pad_shape, pad_value, dtype=arr.dtype)])
        shards.append(np.ascontiguousarray(part))
    return shards


def _shard_feat(arr):
    """[N, D] f32 -> NCORES shards of [NPAD, D] rows (zero-padded tail)."""
    return _shard(arr, 0.0)


def run_with_results(atac_feature, rna_feature, atac_label, rna_label,
                     **run_kwargs):
    from concourse import bass_utils

    nc = _get_nc()
    fa_s = _shard_feat(np.asarray(atac_feature, dtype=np.float32))
    fr_s = _shard_feat(np.asarray(rna_feature, dtype=np.float32))
    la_s = _shard(np.asarray(atac_label, dtype=np.int32), -1)
    lr_s = _shard(np.asarray(rna_label, dtype=np.int32), -1)
    in_maps = [
        {"fa": fa_s[i], "fr": fr_s[i], "la": la_s[i], "lr": lr_s[i]}
        for i in range(NCORES)
    ]
    return bass_utils.run_bass_kernel_spmd(
        nc, in_maps, core_ids=list(range(NCORES)), **run_kwargs)


def _host_tail(sums):
    """Exact [64, 256] InfoNCE tail in float64 on the reduced sums
    (rows 0:64 atac, 64:128 rna)."""
    A = sums[0:K]
    R = sums[K:128]
    A = A / np.maximum(np.sqrt((A * A).sum(1, keepdims=True)), 1e-12)
    R = R / np.maximum(np.sqrt((R * R).sum(1, keepdims=True)), 1e-12)

    Fp = np.exp(A * R / TAU)                               # [K, D]
    Sa = np.exp(A[:, None, :] * A[None, :, :] / TAU)       # [K, K, D]
    Sr = np.exp(A[:, None, :] * R[None, :, :] / TAU)
    off = (1.0 - np.eye(K))[:, :, None]
    Fn = ((Sa + Sr) * off).sum(axis=1) + 2.0 * (K - 1) * Fp
    loss_k = (-np.log(Fp / (Fn + EPS))).mean(axis=1)
    return loss_k.sum()


def kernel(atac_feature, rna_feature, atac_label, rna_label):
    res = run_with_results(atac_feature, rna_feature, atac_label, rna_label)
    sums = np.zeros((128, D), dtype=np.float64)
    for r in res.results:
        sums += np.asarray(r["out"], dtype=np.float64)
    return np.float32(_host_tail(sums))


# revision 23
# speedup vs baseline: 1.1917x; 1.1064x over previous
"""Trainium2 Bass kernel for nn_ContrastiveLoss (prototype InfoNCE loss).

Strategy (data-parallel over the N=100k cell axis, 8 NeuronCores):
  - Each core gets a 12544-row shard (rows padded with label=-1 / feat=0),
    laid out 98 contiguous rows per partition: row = p*98 + j.  Feature
    DMAs then move contiguous multi-KB spans per partition, and the labels
    land in [128, 98] layout directly -- no PE transpose needed.
  - Per tile j, a one-hot [128,64] matrix is built on-chip (DVE is_equal
    against an iota constant) and a bf16 matmul one_hot.T @ feat
    accumulates per-class sums into PSUM ([64, 256], fp32 accumulation).
    Features are cast f32->bf16 in-flight by the SWDGE DMA; the loss is
    insensitive to this rounding.  The stream runs at the per-core HBM
    f32-read roofline (~36 us for 25.7 MB).
  - The stream is ordered ALL-atac-chunks then ALL-rna-chunks, so the
    atac sums finish mid-stream and their PSUM->SBUF copy hides under
    the rna stream.  Post-stream device work is just the rna PSUM copy
    and one 128 KB output DMA.
  - Each core outputs its raw per-class partial sums [128, 256] f32
    (rows 0:64 atac, 64:128 rna).  The host reduces the 8 partials and
    computes the tiny K x K x D InfoNCE on the [64, 256] prototypes in
    float64 -- exact, and off the device critical path entirely (the
    sharding hint's AllReduce is replaced by the host gather that the
    full-I/O contract already requires).  Counts are never materialized:
    l2norm(sums/counts) == sums/||sums||.
"""
import sys

sys.path.insert(0, "/opt/trn_rl_repo")

import numpy as np
from contextlib import ExitStack

N, D, K = 100000, 256, 64
NCORES = 8
NTILES = 98               # tiles of 128 rows per core
NPAD = NTILES * 128       # 12544 rows per core (total 100352 >= 100000)
# Tapered chunk sizes: big chunks amortize DMA overhead; the tail
# shrinks so PE has almost no matmul backlog when the final bytes land.
CHUNKS = [32, 32, 24, 8, 1, 1]
assert sum(CHUNKS) == NTILES
CHMAX = max(CHUNKS)
TAU = 0.5
EPS = 1e-8

_cache = {}


def _build(repeat_main=1, repeat_full=1, feat_fp8=False):
    import concourse.bacc as bacc
    import concourse.tile as tile
    from concourse import mybir

    f32, bf16, i32 = mybir.dt.float32, mybir.dt.bfloat16, mybir.dt.int32
    ft_dt = mybir.dt.float8e4 if feat_fp8 else bf16
    AF = mybir.ActivationFunctionType
    OP = mybir.AluOpType

    nc = bacc.Bacc(None, target_bir_lowering=False, debug=False,
                   num_devices=NCORES)

    fa = nc.dram_tensor("fa", [NPAD, D], f32, kind="ExternalInput")
    fr = nc.dram_tensor("fr", [NPAD, D], f32, kind="ExternalInput")
    la = nc.dram_tensor("la", [NPAD], i32, kind="ExternalInput")
    lr = nc.dram_tensor("lr", [NPAD], i32, kind="ExternalInput")
    # raw per-class partial sums; host reduces across cores and runs the
    # tiny [64, 256] InfoNCE tail in float64
    out = nc.dram_tensor("out", [128, D], f32, kind="ExternalOutput")

    iota_c = nc.inline_tensor(
        np.tile(np.arange(K, dtype=np.float32), (128, 1)), name="iota_c")

    with tile.TileContext(nc) as tc, ExitStack() as ctx:
        consts = ctx.enter_context(tc.tile_pool(name="consts", bufs=1))
        dram = ctx.enter_context(tc.tile_pool(name="dram", bufs=1,
                                              space="DRAM"))

        # iota rides the scalar HWDGE ring so the sync ring's first job
        # is the feature head-start DMA (not this 32 KB constant)
        iota_sb = consts.tile([128, K], f32)
        nc.scalar.dma_start(iota_sb, iota_c[:, :])

        def _body(tk_prev, make_tk):
            with tc.tile_pool(name="fin", bufs=1) as fin, \
                 tc.tile_pool(name="labels", bufs=1) as labels, \
                 tc.tile_pool(name="oh", bufs=1) as ohp, \
                 tc.tile_pool(name="feat", bufs=3) as featp, \
                 tc.tile_pool(name="psum_m", bufs=1, space="PSUM") as psum:

                # Head start: the first two tiles ride HWDGE (sync) as
                # raw f32 -- ~0.6 us first-byte vs SWDGE's ~1.5 us Q7
                # descriptor emission -- so HBM bytes move sooner.  DVE
                # casts them into the bf16 tile off the critical path; the
                # SWDGE chunk covers the rest and emits concurrently.
                HEAD = 2
                ch0 = CHUNKS[0]
                ft0 = featp.tile([128, CHMAX, D], ft_dt,
                                 name="ft_a", tag="ft_a")
                fhead = fin.tile([128, HEAD, D], f32)
                fa_re = fa[:, :].rearrange("(p j) e -> p j e", j=NTILES)
                if tk_prev is not None:
                    # bench-only serializer (repeat_full>1): tiny DMAs
                    # reading rep k's output into the tiles the first real
                    # DMAs overwrite (WAW) -- orders rep k+1's stream
                    # behind rep k's tail.
                    nc.sync.dma_start(fhead[0:1, 0:1, 0:1],
                                      tk_prev[0:1, 0:1])
                    nc.gpsimd.dma_start(ft0[0:1, HEAD, 0:1],
                                        tk_prev[0:1, 0:1])
                nc.sync.dma_start(fhead, fa_re[:, 0:HEAD, :])
                nc.gpsimd.dma_start(ft0[:, HEAD:ch0, :],
                                    fa_re[:, HEAD:ch0, :])
                nc.vector.tensor_copy(ft0[:, 0:HEAD, :], fhead)

                # labels: row p*98+j -> labT[p, j].  Loaded uncast (i32) on
                # the idle SP queue; DVE converts off the critical path.
                labT = {}
                for nm, lab in (("a", la), ("r", lr)):
                    li = labels.tile([128, NTILES], i32, name=f"labi_{nm}")
                    nc.sync.dma_start(
                        li, lab[:].rearrange("(p j) -> p j", j=NTILES))
                    lt = labels.tile([128, NTILES], f32, name=f"labT_{nm}")
                    nc.vector.tensor_copy(lt, li)
                    labT[nm] = lt

                # one-hots: oh[p, t, k] = (label[p*98+t] == k).  atac first
                # (split so chunk0's matmuls start early); rna built in two
                # just-in-time halves.
                oh = {}
                half = NTILES // 2
                for nm in ("a", "r"):
                    o = ohp.tile([128, NTILES, K], ft_dt,
                                 name=f"oh_{nm}")
                    parts = ((0, ch0), (ch0, NTILES)) if nm == "a" else \
                            ((0, half), (half, NTILES))
                    for lo, hi in parts:
                        w = hi - lo
                        nc.vector.tensor_tensor(
                            o[:, lo:hi, :],
                            iota_sb[:, None, :].to_broadcast([128, w, K]),
                            labT[nm][:, lo:hi, None].to_broadcast(
                                [128, w, K]),
                            OP.is_equal,
                        )
                    oh[nm] = o

                # Full-partition PSUM tiles so each accumulator owns its
                # bank at base_partition 0.
                psA_full = psum.tile([128, D], f32)
                psR_full = psum.tile([128, D], f32)
                ps = {"a": psA_full[0:K, :], "r": psR_full[0:K, :]}

                def _stream(nm, feat, first_ft):
                    for rep in range(repeat_main):
                        t0 = 0
                        for ci, w in enumerate(CHUNKS):
                            if ci == 0 and rep == 0 and first_ft is not None:
                                ft = first_ft
                            else:
                                ft = featp.tile([128, CHMAX, D], ft_dt,
                                                name=f"ft_{nm}",
                                                tag=f"ft_{nm}")
                                nc.gpsimd.dma_start(
                                    ft[:, :w, :],
                                    feat[:, :].rearrange(
                                        "(p j) e -> p j e",
                                        j=NTILES)[:, t0:t0 + w, :],
                                )
                            for j in range(w):
                                t = t0 + j
                                nc.tensor.matmul(ps[nm], oh[nm][:, t, :],
                                                 ft[:, j, :],
                                                 start=(t == 0),
                                                 stop=(t == NTILES - 1))
                            t0 += w

                _stream("a", fa, ft0)

                # atac PSUM copy AND its 64 KB out-DMA hide under the rna
                # stream (ACT engine + sync queue)
                outsb = fin.tile([128, D], f32)
                nc.scalar.activation(outsb[0:K, :], ps["a"], AF.Copy)
                nc.sync.dma_start(out[0:K, :], outsb[0:K, :])

                _stream("r", fr, None)

                # post-stream: the rna PSUM copy split across the two
                # idle engines (column halves), each half flushed on its
                # own HWDGE ring (qSPDynamicHW / qActDynamicHW) so the two
                # 32 KB out-DMAs and their completion receipts overlap
                nc.vector.tensor_copy(outsb[K:128, 0:128],
                                      ps["r"][:, 0:128])
                nc.scalar.activation(outsb[K:128, 128:256],
                                     ps["r"][:, 128:256], AF.Copy)
                nc.sync.dma_start(out[K:128, 0:128],
                                  outsb[K:128, 0:128])
                nc.scalar.dma_start(out[K:128, 128:256],
                                    outsb[K:128, 128:256])
                if make_tk:
                    tk = dram.tile([128, 1], f32)
                    nc.sync.dma_start(tk, outsb[:, 0:1])
                    return tk
            return None

        tk_prev = None
        for _full in range(repeat_full):
            tk_prev = _body(tk_prev, make_tk=(repeat_full > 1))

    nc.compile()
    return nc


def _get_nc(repeat_main=1, repeat_full=1, feat_fp8=False):
    key = ("nc", repeat_main, repeat_full, feat_fp8)
    if key not in _cache:
        _cache[key] = _build(repeat_main, repeat_full, feat_fp8)
    return _cache[key]


def _shard(arr, pad_value):
    """Split [N, ...] into NCORES shards of NPAD rows, padding the tail."""
    shards = []
    for i in range(NCORES):
        lo = min(i * NPAD, N)
        hi = min(lo + NPAD, N)
        part = arr[lo:hi]
        if part.shape[0] < NPAD:
            pad_shape = (NPAD - part.shape[0],) + arr.shape[1:]
            part = np.concatenate(
                [part, np.full(pad_shape, pad_value, dtype=arr.dtype)])
        shards.append(np.ascontiguousarray(part))
    return shards


def _shard_feat(arr):
    """[N, D] f32 -> NCORES shards of [NPAD, D] rows (zero-padded tail)."""
    return _shard(arr, 0.0)


def run_with_results(atac_feature, rna_feature, atac_label, rna_label,
                     **run_kwargs):
    from concourse import bass_utils

    nc = _get_nc()
    fa_s = _shard_feat(np.asarray(atac_feature, dtype=np.float32))
    fr_s = _shard_feat(np.asarray(rna_feature, dtype=np.float32))
    la_s = _shard(np.asarray(atac_label, dtype=np.int32), -1)
    lr_s = _shard(np.asarray(rna_label, dtype=np.int32), -1)
    in_maps = [
        {"fa": fa_s[i], "fr": fr_s[i], "la": la_s[i], "lr": lr_s[i]}
        for i in range(NCORES)
    ]
    return bass_utils.run_bass_kernel_spmd(
        nc, in_maps, core_ids=list(range(NCORES)), **run_kwargs)


def _host_tail(sums):
    """Exact [64, 256] InfoNCE tail in float64 on the reduced sums
    (rows 0:64 atac, 64:128 rna)."""
    A = sums[0:K]
    R = sums[K:128]
    A = A / np.maximum(np.sqrt((A * A).sum(1, keepdims=True)), 1e-12)
    R = R / np.maximum(np.sqrt((R * R).sum(1, keepdims=True)), 1e-12)

    Fp = np.exp(A * R / TAU)                               # [K, D]
    Sa = np.exp(A[:, None, :] * A[None, :, :] / TAU)       # [K, K, D]
    Sr = np.exp(A[:, None, :] * R[None, :, :] / TAU)
    off = (1.0 - np.eye(K))[:, :, None]
    Fn = ((Sa + Sr) * off).sum(axis=1) + 2.0 * (K - 1) * Fp
    loss_k = (-np.log(Fp / (Fn + EPS))).mean(axis=1)
    return loss_k.sum()


def kernel(atac_feature, rna_feature, atac_label, rna_label):
    res = run_with_results(atac_feature, rna_feature, atac_label, rna_label)
    sums = np.zeros((128, D), dtype=np.float64)
    for r in res.results:
        sums += np.asarray(r["out"], dtype=np.float64)
    return np.float32(_host_tail(sums))
